# revision 4
# baseline (speedup 1.0000x reference)
"""AttentionSinkPrefill Trainium2 kernel (8 NeuronCores, sequence-parallel).

Module:   Y = AttnSinkPrefill(X) with sink=4, window=256, causal GQA
          (16 q heads, 4 kv heads, head_dim 64, d_model 1024, B=2, T=2048).

Sharding: sequence-parallel over T.  Core c handles queries
          [256c, 256c+256) for both batches.  Because attention is
          sink+window sparse, each core only needs X rows
          [256c-256, 256c+256) (zero-padded at the left boundary) plus the
          sink rows, and computes its o_proj output rows completely --
          no collective and no host-side reduction, outputs concatenate.

Per-core pipeline (single uniform program; per-core variation enters only
through the input data -- padded X slices and masks precomputed on host):
  1. DMA-transpose X window/sink tiles straight into X^T [d_model, keys]
     (bf16, XBAR transpose -- no TensorE transposes needed)
  2. projections (PE):  Q^T = Wq^T X^T (per head), K^T, V (keys-major)
  3. per (head, batch): S^T = K^T^T.T @ Q^T -> exp (ACT) -> multiplicative
     mask (DVE) -> Y^T = V_aug^T P^T where V_aug has a ones column so the
     softmax denominator falls out of the same matmuls -> normalize
  4. O = Y_flat @ Wo  (PE), DMA out

All matmul operands are bf16 (fp32 PSUM accumulation): halves input DMA
bytes, enables XBAR DMA transpose and fast weight load.  Input DMA is
split over the two hardware DGE queues (Sync: X, Scalar: weights/masks).

Host-side tricks: Wq is pre-scaled by 1/sqrt(64) and its columns permuted
(Wo rows likewise) so every q head lands at the same SBUF partition base
as its kv head's K^T rows (matmul requires equal base partitions).
"""

import os
import sys
from contextlib import ExitStack

import numpy as np

sys.path.insert(0, "/opt/trn_rl_repo")

import concourse.bass as bass
import concourse.bacc as bacc
import concourse.mybir as mybir
import concourse.tile as tile
from concourse.bass_utils import run_bass_kernel_spmd

# ---------------------------------------------------------------- constants
D = 1024          # d_model
NH = 16           # q heads
NKV = 4           # kv heads
HD = 64           # head dim
SINK = 4          # attention sink width
SINKP = 16        # sink rows padded to XBAR tile granularity
WIN = 256         # sliding window
B = 2
T = 2048
NCORES = 8
QB = T // NCORES  # queries per core = 256
KW = 2 * QB       # window key rows per core = 512

F32 = mybir.dt.float32
MM_DT = (mybir.dt.float32r if os.environ.get("K_DT", "bf16") == "f32r"
         else mybir.dt.bfloat16)
MM = MM_DT
NP_MM = "float32" if MM_DT == mybir.dt.float32r else "bfloat16"
FR = mybir.dt.float32r  # denominator-broadcast chain stays fp32r
USE_FAST_RECIP = os.environ.get("K_FAST_RECIP", "1") == "1"
PBUFS = int(os.environ.get("K_PBUFS", "2"))
SBUFS = int(os.environ.get("K_SBUFS", "3"))

AF = mybir.ActivationFunctionType

# head order placing each q head at partition base (kv_head%2)*64, paired
# (h, h+4) per 128-feature tile; Wq columns / Wo rows are permuted to match.
HEAD_ORDER = [0, 4, 1, 5, 2, 6, 3, 7, 8, 12, 9, 13, 10, 14, 11, 15]
HEAD_POS = {h: i for i, h in enumerate(HEAD_ORDER)}


# ================================================================ program
def build_nc():
    nc = bacc.Bacc()

    xw_d = nc.dram_tensor("Xw", [B, KW, D], MM, kind="ExternalInput")
    xs_d = nc.dram_tensor("Xs", [B, SINKP, D], MM, kind="ExternalInput")
    wq_d = nc.dram_tensor("Wq", [D, NH * HD], MM, kind="ExternalInput")
    wk_d = nc.dram_tensor("Wk", [D, NKV * HD], MM, kind="ExternalInput")
    wv_d = nc.dram_tensor("Wv", [D, NKV * HD], MM, kind="ExternalInput")
    wo_d = nc.dram_tensor("Wo", [NH * HD, D], MM, kind="ExternalInput")
    mtw_d = nc.dram_tensor("MTw", [128, 4 * QB], MM, kind="ExternalInput")
    mts_d = nc.dram_tensor("MTs", [SINK, QB], MM, kind="ExternalInput")
    zer_d = nc.dram_tensor("ZER", [128, 128], MM, kind="ExternalInput")
    one_d = nc.dram_tensor("ONE", [128, 64], MM, kind="ExternalInput")
    oner_d = nc.dram_tensor("ONER", [128, 64], FR, kind="ExternalInput")
    out_d = nc.dram_tensor("out", [B, QB, D], F32, kind="ExternalOutput")

    KCOL = KW + SINKP  # 528 key columns per batch in X^T layout

    with nc.allow_low_precision(reason="bf16 matmul operands"), \
            tile.TileContext(nc) as tc, ExitStack() as ctx:
        consts = ctx.enter_context(tc.tile_pool(name="consts", bufs=1))
        wpool = ctx.enter_context(tc.tile_pool(name="wpool", bufs=1))
        wopool = ctx.enter_context(tc.tile_pool(name="wop", bufs=1))
        xtp = ctx.enter_context(tc.tile_pool(name="xt", bufs=1))
        qkv = ctx.enter_context(tc.tile_pool(name="qkv", bufs=1))
        ppool = ctx.enter_context(tc.tile_pool(name="pp", bufs=PBUFS))
        ypool = ctx.enter_context(tc.tile_pool(name="yp", bufs=1))
        spool = ctx.enter_context(tc.tile_pool(name="sp", bufs=SBUFS))
        opool = ctx.enter_context(tc.tile_pool(name="op", bufs=2))
        psA = ctx.enter_context(tc.tile_pool(name="psA", bufs=2, space="PSUM"))
        psS = ctx.enter_context(tc.tile_pool(name="psS", bufs=2, space="PSUM"))

        # ---------------- stage 1: X^T straight from DRAM via XBAR DMA
        # transpose.  Sync queue carries X (b-major so batch 0 lands first);
        # Scalar queue carries weights (wk/wv first -- needed first).
        xt = [xtp.tile([128, B * KCOL], MM, tag=f"xt{d}", name=f"xt{d}")
              for d in range(8)]
        for b in range(B):
            for d in range(8):
                nc.sync.dma_start(
                    xt[d][:, b * KCOL:b * KCOL + KW],
                    xw_d[b, :, d * 128:(d + 1) * 128],
                    transpose=True,
                )
        for b in range(B):
            for d in range(8):
                nc.sync.dma_start(
                    xt[d][:, b * KCOL + KW:b * KCOL + KCOL],
                    xs_d[b, :, d * 128:(d + 1) * 128],
                    transpose=True,
                )

        wq = []
        wk = []
        wv = []
        for d in range(8):
            t = wpool.tile([128, NKV * HD], MM, tag=f"wk{d}", name=f"wk{d}")
            nc.scalar.dma_start(t[:], wk_d[d * 128:(d + 1) * 128, :])
            wk.append(t)
        for d in range(8):
            t = wpool.tile([128, NKV * HD], MM, tag=f"wv{d}", name=f"wv{d}")
            nc.scalar.dma_start(t[:], wv_d[d * 128:(d + 1) * 128, :])
            wv.append(t)
        for d in range(8):
            t = wpool.tile([128, NH * HD], MM, tag=f"wq{d}", name=f"wq{d}")
            nc.scalar.dma_start(t[:], wq_d[d * 128:(d + 1) * 128, :])
            wq.append(t)
        mtw = consts.tile([128, 4 * QB], MM, tag="mtw")
        nc.scalar.dma_start(mtw[:], mtw_d[:])
        mts = consts.tile([SINK, QB], MM, tag="mts")
        nc.scalar.dma_start(mts[:], mts_d[:])
        wo = []
        for m in range(8):
            t = wopool.tile([128, D], MM, tag=f"wo{m}", name=f"wo{m}")
            nc.scalar.dma_start(t[:], wo_d[m * 128:(m + 1) * 128, :])
            wo.append(t)

        # persistent per-core tensors
        qt = [qkv.tile([128, B * QB], MM, tag=f"qt{m}", name=f"qt{m}")
              for m in range(8)]
        kt = [qkv.tile([128, B * KW], MM, tag=f"kt{m}", name=f"kt{m}")
              for m in range(2)]
        # zero-padded sink K^T tiles: [feat 128, key 0:4 real | 4:128 zero]
        ktp = {}
        for m in range(2):
            for b in range(B):
                tl = qkv.tile([128, 128], MM, tag=f"ktp{m}{b}", name=f"ktp{m}{b}")
                nc.scalar.dma_start(tl[:], zer_d[:])
                ktp[(m, b)] = tl
        # V in keys-major layout with a ones column per kv head (denominator)
        vt = {}
        for tki in range(4):
            for b in range(B):
                tl = qkv.tile([128, NKV * (HD + 1)], MM,
                              tag=f"vt{tki}{b}", name=f"vt{tki}{b}")
                nc.scalar.dma_start(tl[:, 64:NKV * 65:65], one_d[:, 0:NKV])
                vt[(tki, b)] = tl
        vs = {}
        for b in range(B):
            tl = qkv.tile([SINK, NKV * (HD + 1)], MM, tag=f"vs{b}", name=f"vs{b}")
            nc.scalar.dma_start(tl[0:SINK, 64:NKV * 65:65], one_d[0:SINK, 0:NKV])
            vs[b] = tl
        yt = [ypool.tile([128, B * QB], MM, tag=f"yt{m}", name=f"yt{m}")
              for m in range(8)]
        # ones column used to broadcast the softmax denominator across
        # partitions via a K=1 matmul (row 64 matches ys's denominator row)
        ones = consts.tile([128, 64], FR, tag="ones")
        nc.scalar.dma_start(ones[:], oner_d[:])

        # ---------------- stage 2 per batch: Q/K/V projections
        for b in range(B):
            # K^T: window part and sink part
            for m in range(2):
                ps = psA.tile([128, 512], F32, tag="ys", name=f"kps{b}{m}")
                for d in range(8):
                    nc.tensor.matmul(
                        ps[:],
                        wk[d][:, m * 128:(m + 1) * 128],
                        xt[d][:, b * KCOL:b * KCOL + KW],
                        start=(d == 0), stop=(d == 7),
                    )
                nc.vector.tensor_copy(kt[m][:, b * KW:(b + 1) * KW], ps[:])
                ps2 = psA.tile([128, 512], F32, tag="ys", name=f"ksps{b}{m}")
                for d in range(8):
                    nc.tensor.matmul(
                        ps2[:, 0:SINK],
                        wk[d][:, m * 128:(m + 1) * 128],
                        xt[d][:, b * KCOL + KW:b * KCOL + KW + SINK],
                        start=(d == 0), stop=(d == 7),
                    )
                nc.vector.tensor_copy(ktp[(m, b)][:, 0:SINK], ps2[:, 0:SINK])

            # V (keys-major) + sink V
            for tki in range(4):
                ps = psA.tile([128, 512], F32, tag="ys", name=f"vps{b}{tki}")
                for d in range(8):
                    nc.tensor.matmul(
                        ps[:, 0:NKV * HD],
                        xt[d][:, b * KCOL + tki * 128:b * KCOL + (tki + 1) * 128],
                        wv[d][:],
                        start=(d == 0), stop=(d == 7),
                    )
                for g in range(NKV):
                    nc.vector.tensor_copy(
                        vt[(tki, b)][:, g * 65:g * 65 + HD],
                        ps[:, g * HD:(g + 1) * HD],
                    )
            ps = psA.tile([128, 512], F32, tag="ys", name=f"vsps{b}")
            for d in range(8):
                nc.tensor.matmul(
                    ps[0:SINK, 0:NKV * HD],
                    xt[d][:, b * KCOL + KW:b * KCOL + KW + SINK],
                    wv[d][:],
                    start=(d == 0), stop=(d == 7),
                )
            for g in range(NKV):
                nc.vector.tensor_copy(
                    vs[b][0:SINK, g * 65:g * 65 + HD],
                    ps[0:SINK, g * HD:(g + 1) * HD],
                )

        # Q^T: both batches in one N=512 matmul per (m, d); query columns
        # of X^T sit at offset KW-QB within each batch's KCOL-wide block
        for m in range(8):
            ps = psA.tile([128, 512], F32, tag="ys", name=f"qps{m}")
            for d in range(8):
                rhs = xt[d][:].rearrange(
                    "p (b c) -> p b c", b=B
                )[:, :, KW - QB:KW]
                nc.tensor.matmul(
                    ps[:],
                    wq[d][:, m * 128:(m + 1) * 128],
                    rhs,
                    start=(d == 0), stop=(d == 7),
                )
            nc.vector.tensor_copy(qt[m][:], ps[:])

        # ---------------- stage 3: attention per (batch, head)
        for b in range(B):
            for h in range(NH):
                g = h // 4           # kv head
                mk = g // 2          # K^T tile index
                kb = (g % 2) * 64    # partition base of this kv head's K^T/Q^T
                pos = HEAD_POS[h]
                mq = pos // 2        # Q^T tile index (post-permutation)

                qrhs = qt[mq][kb:kb + 64, b * QB:(b + 1) * QB]

                sp = psS.tile([128, 4 * QB], F32, tag="s", name=f"s{b}{h}")
                ys = psA.tile([128, 512], F32, tag="ys", name=f"ys{b}{h}")
                for tki in range(4):
                    nc.tensor.matmul(
                        sp[:, tki * QB:(tki + 1) * QB],
                        kt[mk][kb:kb + 64, b * KW + tki * 128:b * KW + (tki + 1) * 128],
                        qrhs,
                        start=True, stop=True,
                    )
                nc.tensor.matmul(
                    ys[:, QB:2 * QB],
                    ktp[(mk, b)][kb:kb + 64, :],
                    qrhs,
                    start=True, stop=True,
                )

                p = ppool.tile([128, 5 * QB], MM, tag="p", name=f"p{b}{h}")
                nc.scalar.activation(p[:, 0:4 * QB], sp[:], AF.Exp)
                nc.scalar.activation(p[:, 4 * QB:5 * QB], ys[:, QB:2 * QB], AF.Exp)
                nc.vector.tensor_mul(p[:, 0:4 * QB], p[:, 0:4 * QB], mtw[:])
                nc.vector.tensor_mul(
                    p[0:SINK, 4 * QB:5 * QB], p[0:SINK, 4 * QB:5 * QB], mts[:]
                )

                for tki in range(4):
                    nc.tensor.matmul(
                        ys[0:HD + 1, 0:QB],
                        vt[(tki, b)][:, g * 65:g * 65 + 65],
                        p[:, tki * QB:(tki + 1) * QB],
                        start=(tki == 0), stop=False,
                    )
                nc.tensor.matmul(
                    ys[0:HD + 1, 0:QB],
                    vs[b][0:SINK, g * 65:g * 65 + 65],
                    p[0:SINK, 4 * QB:5 * QB],
                    start=False, stop=True,
                )

                # normalize: row HD of ys is the softmax denominator
                rbp = psA.tile([64, QB], F32, tag="rb", name=f"rbp{b}{h}")
                rb = spool.tile([64, QB], F32, tag="rb", name=f"rb{b}{h}")
                dn = spool.tile([HD + 1, QB], FR, tag="rc", name=f"rc{b}{h}")
                # copy denom to SBUF, broadcast via K=1 matmul, then
                # one reciprocal over the broadcast block
                nc.scalar.copy(dn[HD:HD + 1, :], ys[HD:HD + 1, 0:QB])
                nc.tensor.matmul(
                    rbp[:], ones[HD:HD + 1, :], dn[HD:HD + 1, :],
                    start=True, stop=True,
                )
                if USE_FAST_RECIP:
                    nc.vector.reciprocal_approx_fast(rb[:], rbp[:])
                else:
                    nc.vector.reciprocal(rb[:], rbp[:])
                if kb == 0:
                    nc.vector.tensor_mul(
                        yt[mq][0:64, b * QB:(b + 1) * QB], ys[0:HD, 0:QB], rb[:]
                    )
                else:
                    stg = spool.tile([64, QB], MM, tag="stg", name=f"stg{b}{h}")
                    nc.vector.tensor_mul(stg[:], ys[0:HD, 0:QB], rb[:])
                    nc.sync.dma_start(
                        yt[mq][kb:kb + 64, b * QB:(b + 1) * QB], stg[:]
                    )

        # ---------------- stage 4: O projection
        for b in range(B):
            for mq2 in range(2):
                for nk in range(2):
                    po = psA.tile([128, 512], F32, tag="ys", name=f"po{b}{mq2}{nk}")
                    for m in range(8):
                        nc.tensor.matmul(
                            po[:],
                            yt[m][:, b * QB + mq2 * 128:b * QB + (mq2 + 1) * 128],
                            wo[m][:, nk * 512:(nk + 1) * 512],
                            start=(m == 0), stop=(m == 7),
                        )
                    ost = opool.tile([128, 512], F32, tag="ost", name=f"o{b}{mq2}{nk}")
                    nc.scalar.copy(ost[:], po[:])
                    nc.sync.dma_start(
                        out_d[b, mq2 * 128:(mq2 + 1) * 128, nk * 512:(nk + 1) * 512],
                        ost[:],
                    )

    nc.compile()
    return nc


# ================================================================ host side
def host_prep(X, Wq, Wk, Wv, Wo):
    """Returns in_maps (list of per-core dicts of numpy arrays)."""
    import ml_dtypes
    np_mm = (np.dtype(ml_dtypes.bfloat16) if NP_MM == "bfloat16"
             else np.float32)

    X = np.asarray(X, dtype=np.float32)
    Wq = np.asarray(Wq, dtype=np.float32)
    Wk = np.asarray(Wk, dtype=np.float32)
    Wv = np.asarray(Wv, dtype=np.float32)
    Wo = np.asarray(Wo, dtype=np.float32)

    flat_perm = np.concatenate(
        [np.arange(h * HD, (h + 1) * HD) for h in HEAD_ORDER]
    )
    wq_p = (np.ascontiguousarray(Wq[:, flat_perm])
            * np.float32(1.0 / np.sqrt(HD))).astype(np_mm)
    wo_p = np.ascontiguousarray(Wo[flat_perm, :]).astype(np_mm)
    wk_c = Wk.astype(np_mm)
    wv_c = Wv.astype(np_mm)

    tt = np.arange(T)
    i = tt[:, None]
    j = tt[None, :]
    m_full = (j <= i) & ((j < SINK) | (j >= np.maximum(i - WIN + 1, 0)))
    m_full = m_full.astype(np.float32)

    Xb = X.astype(np_mm)
    xs = np.ascontiguousarray(Xb[:, 0:SINKP, :])

    in_maps = []
    for c in range(NCORES):
        qs = c * QB
        ks = qs - QB  # window starts one query-block earlier (512 rows)

        xw = np.zeros((B, KW, D), dtype=np_mm)
        lo = max(ks, 0)
        xw[:, lo - ks:, :] = Xb[:, lo:ks + KW, :]

        # window mask, transposed: [key 512, query 256] -> [128, 4*256]
        mtw = np.zeros((KW, QB), dtype=np.float32)
        jg = ks + np.arange(KW)
        valid = jg >= 0
        mtw[valid, :] = m_full[qs:qs + QB, jg[valid]].T
        mtw_sb = np.ascontiguousarray(
            mtw.reshape(4, 128, QB).transpose(1, 0, 2).reshape(128, 4 * QB)
        )

        # sink mask [4, 256]; zero where the window tiles already cover col j
        mts = np.zeros((SINK, QB), dtype=np.float32)
        for jj in range(SINK):
            if not (ks <= jj < ks + KW):
                mts[jj, :] = m_full[qs:qs + QB, jj]

        in_maps.append({
            "ZER": np.zeros((128, 128), dtype=np_mm),
            "ONE": np.ones((128, 64), dtype=np_mm),
            "ONER": np.ones((128, 64), dtype=np.float32),
            "Xw": xw,
            "Xs": xs,
            "Wq": wq_p,
            "Wk": wk_c,
            "Wv": wv_c,
            "Wo": wo_p,
            "MTw": mtw_sb.astype(np_mm),
            "MTs": mts.astype(np_mm),
        })
    return in_maps


_NC_CACHE = {}


def get_nc():
    if "nc" not in _NC_CACHE:
        _NC_CACHE["nc"] = build_nc()
    return _NC_CACHE["nc"]


def kernel(X, Wq, Wk, Wv, Wo):
    in_maps = host_prep(X, Wq, Wk, Wv, Wo)
    nc = get_nc()
    res = run_bass_kernel_spmd(nc, in_maps, list(range(NCORES)))
    out = np.empty((B, T, D), dtype=np.float32)
    for c in range(NCORES):
        out[:, c * QB:(c + 1) * QB, :] = res.results[c]["out"]
    return out


# revision 10
# speedup vs baseline: 1.3378x; 1.3378x over previous
"""AttentionSinkPrefill Trainium2 kernel (8 NeuronCores, sequence-parallel).

Module:   Y = AttnSinkPrefill(X) with sink=4, window=256, causal GQA
          (16 q heads, 4 kv heads, head_dim 64, d_model 1024, B=2, T=2048).

Sharding: sequence-parallel over T.  Core c handles queries
          [256c, 256c+256) for both batches; the sink+window structure
          means it only needs X rows [256c-256, 256c+256) plus the 4 sink
          rows, computes its o_proj output rows completely -- no
          collective, outputs concatenate.

Per-core pipeline (bf16 operands, fp32 PSUM accumulation):
  0. 32 warm-up matmuls on a zero tile (HAM un-throttle: PE 1.2->2.4GHz)
  1. X^T [d_model, keys] lands straight from DRAM via XBAR DMA transpose
  2. projections (PE): K^T, V (keys-major), Q^T; weights arrive as one
     contiguous [128, 8*cols] block each (single big-packet DMA)
  3. per (batch, head-pair): 10 score MMs -> bf16 PSUM scores tile
     [128, 2560] -> one exp (ACT) -> one mask multiply (DVE/GpSimd
     alternating) -> 10 PV MMs where V_aug = [V | ones*64] so the PV
     matmul itself writes the softmax denominator broadcast across
     partitions 64:128 (matmul cost is N-cycles, M is free) -> one
     reciprocal + two normalize multiplies
  4. O = Y_flat @ Wo  (PE), DMA out

Sink K/V (4 rows, 0.01% of FLOPs) are precomputed on host like the
masks.  Wq is pre-scaled by 1/sqrt(64); Wq columns / Wo rows permuted so
every q head lands at the partition base of its kv head's K^T rows.
"""

import os
import sys
from contextlib import ExitStack

import numpy as np

sys.path.insert(0, "/opt/trn_rl_repo")

import concourse.bass as bass
import concourse.bacc as bacc
import concourse.mybir as mybir
import concourse.tile as tile
from concourse.bass_utils import run_bass_kernel_spmd

# ---------------------------------------------------------------- constants
D = 1024          # d_model
NH = 16           # q heads
NKV = 4           # kv heads
HD = 64           # head dim
SINK = 4          # attention sink width
WIN = 256         # sliding window
B = 2
T = 2048
NCORES = 8
QB = T // NCORES  # queries per core = 256
KW = 2 * QB       # window key rows per core = 512
SPW = 10 * QB     # p-tile cols per pair: 2 heads x (4 win + 1 sink) x 256
SPWIN = 8 * QB    # window score PSUM tile cols per pair

F32 = mybir.dt.float32
MM = mybir.dt.bfloat16
USE_FAST_RECIP = os.environ.get("K_FAST_RECIP", "1") == "1"
MASK_GPS = os.environ.get("K_MASK_GPS", "alt")  # 'dve' | 'gps' | 'alt'
NWARM = int(os.environ.get("K_NWARM", "32"))
PBUFS = int(os.environ.get("K_PBUFS", "2"))
SBUFS = int(os.environ.get("K_SBUFS", "3"))

AF = mybir.ActivationFunctionType

# head order placing each q head at partition base (kv_head%2)*64, paired
# (h, h+4) per 128-feature tile; Wq columns / Wo rows are permuted to match.
HEAD_ORDER = [0, 4, 1, 5, 2, 6, 3, 7, 8, 12, 9, 13, 10, 14, 11, 15]
HEAD_POS = {h: i for i, h in enumerate(HEAD_ORDER)}


# ================================================================ program
def build_nc():
    nc = bacc.Bacc()

    xw_d = nc.dram_tensor("Xw", [B, KW, D], MM, kind="ExternalInput")
    wq_d = nc.dram_tensor("Wq", [128, 8 * NH * HD], MM, kind="ExternalInput")
    wk_d = nc.dram_tensor("Wk", [128, 8 * NKV * HD], MM, kind="ExternalInput")
    wv_d = nc.dram_tensor("Wv", [128, 8 * NKV * HD], MM, kind="ExternalInput")
    wo_d = nc.dram_tensor("Wo", [128, 8 * D], MM, kind="ExternalInput")
    mtw_d = nc.dram_tensor("MTw", [128, SPW], MM, kind="ExternalInput")
    ktp_d = nc.dram_tensor("KTP", [B, 2, 128, SINK], MM, kind="ExternalInput")
    vs_d = nc.dram_tensor("VS", [B, SINK, NKV * 128], MM, kind="ExternalInput")
    zer_d = nc.dram_tensor("ZER", [128, 128], MM, kind="ExternalInput")
    oneb_d = nc.dram_tensor("ONEB", [128, 4 * NKV * HD], MM, kind="ExternalInput")
    oner_d = nc.dram_tensor("ONER", [128, 64], mybir.dt.float32r, kind="ExternalInput")
    out_d = nc.dram_tensor("out", [B, QB, D], F32, kind="ExternalOutput")

    with nc.allow_low_precision(reason="bf16 matmul operands"), \
            tile.TileContext(nc) as tc, ExitStack() as ctx:
        consts = ctx.enter_context(tc.tile_pool(name="consts", bufs=1))
        wpool = ctx.enter_context(tc.tile_pool(name="wpool", bufs=1))
        xtp = ctx.enter_context(tc.tile_pool(name="xt", bufs=1))
        qkv = ctx.enter_context(tc.tile_pool(name="qkv", bufs=1))
        ppool = ctx.enter_context(tc.tile_pool(name="pp", bufs=PBUFS))
        ypool = ctx.enter_context(tc.tile_pool(name="yp", bufs=1))
        spool = ctx.enter_context(tc.tile_pool(name="sp", bufs=SBUFS))
        opool = ctx.enter_context(tc.tile_pool(name="op", bufs=2))
        psA = ctx.enter_context(tc.tile_pool(name="psA", bufs=2, space="PSUM"))
        psS = ctx.enter_context(tc.tile_pool(name="psS", bufs=1, space="PSUM"))
        psB = ctx.enter_context(tc.tile_pool(name="psB", bufs=2, space="PSUM"))

        # ---------------- stage 0: HAM warm-up (PE clock 1.2 -> 2.4 GHz)
        warmz = consts.tile([128, 128], MM, tag="warmz")
        nc.sync.dma_start(warmz[:], zer_d[:])
        for w in range(NWARM):
            wps = psA.tile([128, 512], F32, tag="ys", name=f"warm{w}")
            nc.tensor.matmul(wps[:, 0:128], warmz[:], warmz[:],
                             start=True, stop=True)

        # ---------------- stage 1: X^T via XBAR DMA transpose, one instr
        # per d-chunk covering both batches ([1024, 128] -> [128, 1024]).
        # Split across the two HW DGE queues (Sync / Scalar).
        xt = [xtp.tile([128, B * KW], MM, tag=f"xt{d}", name=f"xt{d}")
              for d in range(8)]
        for d in range(8):
            eng = nc.sync if d % 2 == 0 else nc.scalar
            eng.dma_start(
                xt[d][:],
                xw_d[:, :, d * 128:(d + 1) * 128].rearrange("b r c -> (b r) c"),
                transpose=True,
            )

        # weights: single contiguous big-packet DMA each
        wkt = wpool.tile([128, 8 * NKV * HD], MM, tag="wk")
        nc.scalar.dma_start(wkt[:], wk_d[:])
        wvt = wpool.tile([128, 8 * NKV * HD], MM, tag="wv")
        nc.scalar.dma_start(wvt[:], wv_d[:])
        wqt = wpool.tile([128, 8 * NH * HD], MM, tag="wq")
        nc.sync.dma_start(wqt[:], wq_d[:])
        mtw = consts.tile([128, SPW], MM, tag="mtw")
        nc.scalar.dma_start(mtw[:], mtw_d[:])
        wot = wpool.tile([128, 8 * D], MM, tag="wo")
        nc.scalar.dma_start(wot[:], wo_d[:])

        # persistent per-core tensors
        qt = [qkv.tile([128, B * QB], MM, tag=f"qt{m}", name=f"qt{m}")
              for m in range(8)]
        kt = [qkv.tile([128, B * KW], MM, tag=f"kt{m}", name=f"kt{m}")
              for m in range(2)]
        # zero-padded sink K^T tiles (host-computed): [128, 4 real | 124 zero]
        ktp = {}
        for m in range(2):
            for b in range(B):
                tl = qkv.tile([128, 128], MM, tag=f"ktp{m}{b}", name=f"ktp{m}{b}")
                nc.sync.dma_start(tl[:, 0:SINK], ktp_d[b, m])
                nc.sync.dma_start(tl[:, SINK:128], zer_d[:, SINK:128])
                ktp[(m, b)] = tl
        # V_aug keys-major per batch: [128, tki*512 + g*128 + (V 0:64|ones 64:128)]
        vtb = []
        for b in range(B):
            tl = qkv.tile([128, 4 * NKV * 128], MM, tag=f"vt{b}", name=f"vt{b}")
            dst = tl[:].rearrange("p (t g w c) -> p (t g) w c", t=4, g=NKV, w=2)
            nc.scalar.dma_start(dst[:, :, 1, :], oneb_d[:])
            vtb.append(tl)
        # host-computed sink V_aug: ones included host-side
        vs = {}
        for b in range(B):
            tl = qkv.tile([SINK, NKV * 128], MM, tag=f"vs{b}", name=f"vs{b}")
            nc.sync.dma_start(tl[:], vs_d[b])
            vs[b] = tl
        yt = [ypool.tile([128, B * QB], MM, tag=f"yt{m}", name=f"yt{m}")
              for m in range(8)]
        # fp32r ones column for the denominator-broadcast K=1 matmul (row 64)
        ones = consts.tile([128, 64], mybir.dt.float32r, tag="ones")
        nc.sync.dma_start(ones[:], oner_d[:])

        # ---------------- stage 2 per batch: K/V projections, then Q
        for b in range(B):
            for m in range(2):
                ps = psA.tile([128, 512], F32, tag="ys", name=f"kps{b}{m}")
                for d in range(8):
                    nc.tensor.matmul(
                        ps[:],
                        wkt[:, d * 256 + m * 128:d * 256 + (m + 1) * 128],
                        xt[d][:, b * KW:(b + 1) * KW],
                        start=(d == 0), stop=(d == 7),
                    )
                nc.vector.tensor_copy(kt[m][:, b * KW:(b + 1) * KW], ps[:])

            for tki in range(4):
                ps = psA.tile([128, 512], F32, tag="ys", name=f"vps{b}{tki}")
                for d in range(8):
                    nc.tensor.matmul(
                        ps[:, 0:NKV * HD],
                        xt[d][:, b * KW + tki * 128:b * KW + (tki + 1) * 128],
                        wvt[:, d * 256:(d + 1) * 256],
                        start=(d == 0), stop=(d == 7),
                    )
                for g in range(NKV):
                    nc.vector.tensor_copy(
                        vtb[b][:, tki * 512 + g * 128:tki * 512 + g * 128 + HD],
                        ps[:, g * HD:(g + 1) * HD],
                    )

        # Q^T: both batches in one N=512 matmul per (m, d); query columns
        # of X^T sit at cols [QB, 2*QB) within each batch's KW-wide block
        for m in range(8):
            ps = psA.tile([128, 512], F32, tag="ys", name=f"qps{m}")
            for d in range(8):
                rhs = xt[d][:].rearrange(
                    "p (b c) -> p b c", b=B
                )[:, :, KW - QB:KW]
                nc.tensor.matmul(
                    ps[:],
                    wqt[:, d * 1024 + m * 128:d * 1024 + (m + 1) * 128],
                    rhs,
                    start=(d == 0), stop=(d == 7),
                )
            nc.vector.tensor_copy(qt[m][:], ps[:])

        # ---------------- stage 3: attention per (batch, head-pair)
        pair_i = 0
        for b in range(B):
            for g in range(NKV):
                mk = g // 2          # K^T tile index
                kb = (g % 2) * 64    # partition base of this kv head
                for e2 in range(2):
                    h0 = 4 * g + 2 * e2
                    mqs = [HEAD_POS[h0] // 2, HEAD_POS[h0 + 1] // 2]

                    # window scores PSUM [128, 2048]; sink scores in snk
                    sp = psS.tile([128, SPWIN], F32, tag="s", name=f"s{b}{g}{e2}")
                    snk = psB.tile([128, 512], F32, tag="k", name=f"k{b}{g}{e2}")
                    for e in range(2):
                        qrhs = qt[mqs[e]][kb:kb + 64, b * QB:(b + 1) * QB]
                        for tki in range(4):
                            nc.tensor.matmul(
                                sp[:, tki * 512 + e * QB:tki * 512 + (e + 1) * QB],
                                kt[mk][kb:kb + 64,
                                       b * KW + tki * 128:b * KW + (tki + 1) * 128],
                                qrhs,
                                start=True, stop=True,
                            )
                        nc.tensor.matmul(
                            snk[:, e * QB:(e + 1) * QB],
                            ktp[(mk, b)][kb:kb + 64, :],
                            qrhs,
                            start=True, stop=True,
                        )

                    p = ppool.tile([128, SPW], MM, tag="p", name=f"p{b}{g}{e2}")
                    nc.scalar.activation(p[:, 0:SPWIN], sp[:], AF.Exp)
                    nc.scalar.activation(p[:, SPWIN:SPW], snk[:], AF.Exp)
                    if MASK_GPS == "gps" or (MASK_GPS == "alt" and pair_i % 2 == 0):
                        nc.gpsimd.tensor_mul(p[:], p[:], mtw[:])
                    else:
                        nc.vector.tensor_mul(p[:], p[:], mtw[:])

                    # PV: V_aug = [V | ones*64] -> rows 0:64 Y, 64:128 denom
                    ys = psA.tile([128, 512], F32, tag="ys", name=f"ys{b}{g}{e2}")
                    for e in range(2):
                        for tki in range(4):
                            nc.tensor.matmul(
                                ys[:, e * QB:(e + 1) * QB],
                                vtb[b][:, tki * 512 + g * 128:tki * 512 + (g + 1) * 128],
                                p[:, tki * 512 + e * QB:tki * 512 + (e + 1) * QB],
                                start=(tki == 0), stop=False,
                            )
                        nc.tensor.matmul(
                            ys[:, e * QB:(e + 1) * QB],
                            vs[b][0:SINK, g * 128:(g + 1) * 128],
                            p[0:SINK, 8 * QB + e * QB:8 * QB + (e + 1) * QB],
                            start=False, stop=True,
                        )

                    # denominator: row 64 of ys -> SBUF (same-base copy) ->
                    # K=1 matmul broadcasts it to partitions 0:64 (into the
                    # recycled snk bank) -> reciprocal at base 0
                    dn = spool.tile([128, 512], mybir.dt.float32r, tag="dn",
                                    name=f"dn{b}{g}{e2}")
                    nc.vector.tensor_copy(dn[64:65, :], ys[64:65, :])
                    nc.tensor.matmul(
                        snk[0:64, :], ones[64:65, :], dn[64:65, :],
                        start=True, stop=True,
                    )
                    rb = spool.tile([64, 512], F32, tag="rb", name=f"rb{b}{g}{e2}")
                    if USE_FAST_RECIP:
                        nc.vector.reciprocal_approx_fast(rb[:], snk[0:64, :])
                    else:
                        nc.vector.reciprocal(rb[:], snk[0:64, :])
                    for e in range(2):
                        mq = mqs[e]
                        if kb == 0:
                            nc.vector.tensor_mul(
                                yt[mq][0:64, b * QB:(b + 1) * QB],
                                ys[0:HD, e * QB:(e + 1) * QB],
                                rb[:, e * QB:(e + 1) * QB],
                            )
                        else:
                            stg = spool.tile([64, QB], MM, tag=f"stg{e}",
                                             name=f"stg{b}{g}{e2}{e}")
                            nc.vector.tensor_mul(
                                stg[:],
                                ys[0:HD, e * QB:(e + 1) * QB],
                                rb[:, e * QB:(e + 1) * QB],
                            )
                            nc.sync.dma_start(
                                yt[mq][kb:kb + 64, b * QB:(b + 1) * QB], stg[:]
                            )
                    pair_i += 1

        # ---------------- stage 4: O projection
        for b in range(B):
            for mq2 in range(2):
                for nk in range(2):
                    po = psA.tile([128, 512], F32, tag="ys", name=f"po{b}{mq2}{nk}")
                    for m in range(8):
                        nc.tensor.matmul(
                            po[:],
                            yt[m][:, b * QB + mq2 * 128:b * QB + (mq2 + 1) * 128],
                            wot[:, m * 1024 + nk * 512:m * 1024 + (nk + 1) * 512],
                            start=(m == 0), stop=(m == 7),
                        )
                    ost = opool.tile([128, 512], F32, tag="ost", name=f"o{b}{mq2}{nk}")
                    nc.vector.tensor_copy(ost[:], po[:])
                    nc.sync.dma_start(
                        out_d[b, mq2 * 128:(mq2 + 1) * 128, nk * 512:(nk + 1) * 512],
                        ost[:],
                    )

    nc.compile()
    return nc


# ================================================================ host side
def host_prep(X, Wq, Wk, Wv, Wo):
    """Returns in_maps (list of per-core dicts of numpy arrays)."""
    import ml_dtypes
    np_mm = np.dtype(ml_dtypes.bfloat16)

    X = np.asarray(X, dtype=np.float32)
    Wq = np.asarray(Wq, dtype=np.float32)
    Wk = np.asarray(Wk, dtype=np.float32)
    Wv = np.asarray(Wv, dtype=np.float32)
    Wo = np.asarray(Wo, dtype=np.float32)

    flat_perm = np.concatenate(
        [np.arange(h * HD, (h + 1) * HD) for h in HEAD_ORDER]
    )
    wq_p = (np.ascontiguousarray(Wq[:, flat_perm])
            * np.float32(1.0 / np.sqrt(HD)))
    wo_p = np.ascontiguousarray(Wo[flat_perm, :])

    # pack weights into [128, 8*cols]: partition p col-block d = rows d*128+p
    def pack(w):
        dd, cc = w.shape
        return np.ascontiguousarray(
            w.reshape(8, 128, cc).transpose(1, 0, 2).reshape(128, 8 * cc)
        ).astype(np_mm)

    wq_sb = pack(wq_p)
    wk_sb = pack(Wk)
    wv_sb = pack(Wv)
    wo_sb = pack(wo_p)

    # sink K^T / V_aug (host-computed, like the masks)
    Xs = X[:, 0:SINK, :]                       # [B, 4, D]
    Ks = Xs @ Wk                               # [B, 4, 256]
    Vsk = Xs @ Wv                              # [B, 4, 256]
    ktp_h = np.zeros((B, 2, 128, SINK), dtype=np.float32)
    vs_h = np.zeros((B, SINK, NKV * 128), dtype=np.float32)
    for b in range(B):
        for m in range(2):
            ktp_h[b, m] = Ks[b][:, m * 128:(m + 1) * 128].T
        for g in range(NKV):
            vs_h[b, :, g * 128:g * 128 + HD] = Vsk[b][:, g * HD:(g + 1) * HD]
            vs_h[b, :, g * 128 + HD:(g + 1) * 128] = 1.0

    tt = np.arange(T)
    i = tt[:, None]
    j = tt[None, :]
    m_full = (j <= i) & ((j < SINK) | (j >= np.maximum(i - WIN + 1, 0)))
    m_full = m_full.astype(np.float32)

    Xb = X.astype(np_mm)

    in_maps = []
    for c in range(NCORES):
        qs = c * QB
        ks = qs - QB  # window starts one query-block earlier (512 rows)

        xw = np.zeros((B, KW, D), dtype=np_mm)
        lo = max(ks, 0)
        xw[:, lo - ks:, :] = Xb[:, lo:ks + KW, :]

        # window mask, transposed: [key 512, query 256] -> [128, 4*256]
        mtw = np.zeros((KW, QB), dtype=np.float32)
        jg = ks + np.arange(KW)
        valid = jg >= 0
        mtw[valid, :] = m_full[qs:qs + QB, jg[valid]].T

        # sink mask [4, 256]; zero where the window tiles already cover col j
        mts = np.zeros((SINK, QB), dtype=np.float32)
        for jj in range(SINK):
            if not (ks <= jj < ks + KW):
                mts[jj, :] = m_full[qs:qs + QB, jj]

        # pair-merged mask [128, 10*QB]: cols tki*512 + e*256 + q (window,
        # same for both heads), 2048 + e*256 + q (sink, rows 4:128 zero)
        mtw2 = np.zeros((128, SPW), dtype=np.float32)
        wm = mtw.reshape(4, 128, QB)
        for tki in range(4):
            for e in range(2):
                mtw2[:, tki * 512 + e * QB:tki * 512 + (e + 1) * QB] = wm[tki]
        for e in range(2):
            mtw2[0:SINK, 8 * QB + e * QB:8 * QB + (e + 1) * QB] = mts

        in_maps.append({
            "ZER": np.zeros((128, 128), dtype=np_mm),
            "ONER": np.ones((128, 64), dtype=np.float32),
            "ONEB": np.ones((128, 4 * NKV * HD), dtype=np_mm),
            "Xw": xw,
            "Wq": wq_sb,
            "Wk": wk_sb,
            "Wv": wv_sb,
            "Wo": wo_sb,
            "MTw": mtw2.astype(np_mm),
            "KTP": ktp_h.astype(np_mm),
            "VS": vs_h.astype(np_mm),
        })
    return in_maps


_NC_CACHE = {}


def get_nc():
    if "nc" not in _NC_CACHE:
        _NC_CACHE["nc"] = build_nc()
    return _NC_CACHE["nc"]


def kernel(X, Wq, Wk, Wv, Wo):
    in_maps = host_prep(X, Wq, Wk, Wv, Wo)
    nc = get_nc()
    res = run_bass_kernel_spmd(nc, in_maps, list(range(NCORES)))
    out = np.empty((B, T, D), dtype=np.float32)
    for c in range(NCORES):
        out[:, c * QB:(c + 1) * QB, :] = res.results[c]["out"]
    return out


# revision 12
# speedup vs baseline: 1.5064x; 1.1260x over previous
"""AttentionSinkPrefill Trainium2 kernel (8 NeuronCores, sequence-parallel).

Module:   Y = AttnSinkPrefill(X) with sink=4, window=256, causal GQA
          (16 q heads, 4 kv heads, head_dim 64, d_model 1024, B=2, T=2048).

Sharding: sequence-parallel over T.  Core c handles queries
          [256c, 256c+256) for both batches; the sink+window structure
          means it only needs X rows [256c-256, 256c+256) plus the 4 sink
          rows, computes its o_proj output rows completely -- no
          collective, outputs concatenate.

Per-core pipeline (bf16 operands, fp32 PSUM accumulation):
  0. 32 warm-up matmuls on a zero tile (HAM un-throttle: PE 1.2->2.4GHz)
  1. X^T [d_model, keys] lands straight from DRAM via XBAR DMA transpose
  2. projections (PE): K^T, V (keys-major), Q^T; weights arrive as one
     contiguous [128, 8*cols] block each (single big-packet DMA)
  3. per (batch, head-pair): 10 score MMs -> bf16 PSUM scores tile
     [128, 2560] -> one exp (ACT) -> one mask multiply (DVE/GpSimd
     alternating) -> 10 PV MMs where V_aug = [V | ones*64] so the PV
     matmul itself writes the softmax denominator broadcast across
     partitions 64:128 (matmul cost is N-cycles, M is free) -> one
     reciprocal + two normalize multiplies
  4. O = Y_flat @ Wo  (PE), DMA out

Sink K/V (4 rows, 0.01% of FLOPs) are precomputed on host like the
masks.  Wq is pre-scaled by 1/sqrt(64); Wq columns / Wo rows permuted so
every q head lands at the partition base of its kv head's K^T rows.
"""

import os
import sys
from contextlib import ExitStack

import numpy as np

sys.path.insert(0, "/opt/trn_rl_repo")

import concourse.bass as bass
import concourse.bacc as bacc
import concourse.mybir as mybir
import concourse.tile as tile
from concourse.bass_utils import run_bass_kernel_spmd

# ---------------------------------------------------------------- constants
D = 1024          # d_model
NH = 16           # q heads
NKV = 4           # kv heads
HD = 64           # head dim
SINK = 4          # attention sink width
WIN = 256         # sliding window
B = 2
T = 2048
NCORES = 8
QB = T // NCORES  # queries per core = 256
KW = 2 * QB       # window key rows per core = 512
SPW = 10 * QB     # p-tile cols per pair: 2 heads x (4 win + 1 sink) x 256
SPWIN = 8 * QB    # window score PSUM tile cols per pair

F32 = mybir.dt.float32
MM = mybir.dt.bfloat16
USE_FAST_RECIP = os.environ.get("K_FAST_RECIP", "1") == "1"
MASK_GPS = os.environ.get("K_MASK_GPS", "alt")  # 'dve' | 'gps' | 'alt'
NWARM = int(os.environ.get("K_NWARM", "64"))
PBUFS = int(os.environ.get("K_PBUFS", "3"))
SBUFS = int(os.environ.get("K_SBUFS", "3"))

AF = mybir.ActivationFunctionType

# head order placing each q head at partition base (kv_head%2)*64, paired
# (h, h+4) per 128-feature tile; Wq columns / Wo rows are permuted to match.
HEAD_ORDER = [0, 4, 1, 5, 2, 6, 3, 7, 8, 12, 9, 13, 10, 14, 11, 15]
HEAD_POS = {h: i for i, h in enumerate(HEAD_ORDER)}


# ================================================================ program
def build_nc():
    nc = bacc.Bacc()

    xw_d = nc.dram_tensor("Xw", [B, KW, D], MM, kind="ExternalInput")
    wq_d = nc.dram_tensor("Wq", [128, 8 * NH * HD], MM, kind="ExternalInput")
    wk_d = nc.dram_tensor("Wk", [128, 8 * NKV * HD], MM, kind="ExternalInput")
    wv_d = nc.dram_tensor("Wv", [128, 8 * NKV * HD], MM, kind="ExternalInput")
    wo_d = nc.dram_tensor("Wo", [128, 8 * D], MM, kind="ExternalInput")
    mtw_d = nc.dram_tensor("MTw", [128, SPW], MM, kind="ExternalInput")
    ktp_d = nc.dram_tensor("KTP", [B, 2, 128, SINK], MM, kind="ExternalInput")
    vs_d = nc.dram_tensor("VS", [B, SINK, NKV * 128], MM, kind="ExternalInput")
    zer_d = nc.dram_tensor("ZER", [128, 128], MM, kind="ExternalInput")
    oneb_d = nc.dram_tensor("ONEB", [128, 4 * NKV * HD], MM, kind="ExternalInput")
    oner_d = nc.dram_tensor("ONER", [128, 64], mybir.dt.float32r, kind="ExternalInput")
    out_d = nc.dram_tensor("out", [B, QB, D], MM, kind="ExternalOutput")

    with nc.allow_low_precision(reason="bf16 matmul operands"), \
            tile.TileContext(nc) as tc, ExitStack() as ctx:
        consts = ctx.enter_context(tc.tile_pool(name="consts", bufs=1))
        wpool = ctx.enter_context(tc.tile_pool(name="wpool", bufs=1))
        xtp = ctx.enter_context(tc.tile_pool(name="xt", bufs=1))
        qkv = ctx.enter_context(tc.tile_pool(name="qkv", bufs=1))
        ppool = ctx.enter_context(tc.tile_pool(name="pp", bufs=PBUFS))
        ypool = ctx.enter_context(tc.tile_pool(name="yp", bufs=1))
        spool = ctx.enter_context(tc.tile_pool(name="sp", bufs=SBUFS))
        opool = ctx.enter_context(tc.tile_pool(name="op", bufs=2))
        psA = ctx.enter_context(tc.tile_pool(name="psA", bufs=2, space="PSUM"))
        psS = ctx.enter_context(tc.tile_pool(name="psS", bufs=1, space="PSUM"))
        psB = ctx.enter_context(tc.tile_pool(name="psB", bufs=2, space="PSUM"))

        # ---------------- stage 0: HAM warm-up (PE clock 1.2 -> 2.4 GHz)
        warmz = consts.tile([128, 128], MM, tag="warmz")
        nc.sync.dma_start(warmz[:], zer_d[:])
        for w in range(NWARM):
            wps = psA.tile([128, 512], F32, tag="ys", name=f"warm{w}")
            nc.tensor.matmul(wps[:, 0:128], warmz[:], warmz[:],
                             start=True, stop=True)

        # ---------------- stage 1: X^T via XBAR DMA transpose, one instr
        # per d-chunk covering both batches ([1024, 128] -> [128, 1024]).
        # Split across the two HW DGE queues (Sync / Scalar).
        xt = [xtp.tile([128, B * KW], MM, tag=f"xt{d}", name=f"xt{d}")
              for d in range(8)]
        for d in range(8):
            eng = nc.sync if d % 2 == 0 else nc.scalar
            eng.dma_start(
                xt[d][:],
                xw_d[:, :, d * 128:(d + 1) * 128].rearrange("b r c -> (b r) c"),
                transpose=True,
            )

        # weights: single contiguous big-packet DMA each
        wkt = wpool.tile([128, 8 * NKV * HD], MM, tag="wk")
        nc.scalar.dma_start(wkt[:], wk_d[:])
        wvt = wpool.tile([128, 8 * NKV * HD], MM, tag="wv")
        nc.scalar.dma_start(wvt[:], wv_d[:])
        wqt = wpool.tile([128, 8 * NH * HD], MM, tag="wq")
        nc.sync.dma_start(wqt[:], wq_d[:])
        mtw = consts.tile([128, SPW], MM, tag="mtw")
        nc.scalar.dma_start(mtw[:], mtw_d[:])
        wot = wpool.tile([128, 8 * D], MM, tag="wo")
        nc.scalar.dma_start(wot[:], wo_d[:])

        # persistent per-core tensors
        qt = [qkv.tile([128, B * QB], MM, tag=f"qt{m}", name=f"qt{m}")
              for m in range(8)]
        kt = [qkv.tile([128, B * KW], MM, tag=f"kt{m}", name=f"kt{m}")
              for m in range(2)]
        # zero-padded sink K^T tiles (host-computed): [128, 4 real | 124 zero]
        ktp = {}
        for m in range(2):
            for b in range(B):
                tl = qkv.tile([128, 128], MM, tag=f"ktp{m}{b}", name=f"ktp{m}{b}")
                nc.sync.dma_start(tl[:, 0:SINK], ktp_d[b, m])
                nc.sync.dma_start(tl[:, SINK:128], zer_d[:, SINK:128])
                ktp[(m, b)] = tl
        # V_aug keys-major per batch: [128, tki*512 + g*128 + (V 0:64|ones 64:128)]
        vtb = []
        for b in range(B):
            tl = qkv.tile([128, 4 * NKV * 128], MM, tag=f"vt{b}", name=f"vt{b}")
            dst = tl[:].rearrange("p (t g w c) -> p (t g) w c", t=4, g=NKV, w=2)
            nc.scalar.dma_start(dst[:, :, 1, :], oneb_d[:])
            vtb.append(tl)
        # host-computed sink V_aug: ones included host-side
        vs = {}
        for b in range(B):
            tl = qkv.tile([SINK, NKV * 128], MM, tag=f"vs{b}", name=f"vs{b}")
            nc.sync.dma_start(tl[:], vs_d[b])
            vs[b] = tl
        yt = [ypool.tile([128, B * QB], MM, tag=f"yt{m}", name=f"yt{m}")
              for m in range(8)]
        # fp32r ones column for the denominator-broadcast K=1 matmul (row 64)
        ones = consts.tile([128, 64], mybir.dt.float32r, tag="ones")
        nc.sync.dma_start(ones[:], oner_d[:])

        # ---------------- stage 2 per batch: K/V projections, then Q
        for b in range(B):
            for m in range(2):
                ps = psA.tile([128, 512], F32, tag="ys", name=f"kps{b}{m}")
                for d in range(8):
                    nc.tensor.matmul(
                        ps[:],
                        wkt[:, d * 256 + m * 128:d * 256 + (m + 1) * 128],
                        xt[d][:, b * KW:(b + 1) * KW],
                        start=(d == 0), stop=(d == 7),
                    )
                nc.vector.tensor_copy(kt[m][:, b * KW:(b + 1) * KW], ps[:])

            for tki in range(4):
                ps = psA.tile([128, 512], F32, tag="ys", name=f"vps{b}{tki}")
                for d in range(8):
                    nc.tensor.matmul(
                        ps[:, 0:NKV * HD],
                        xt[d][:, b * KW + tki * 128:b * KW + (tki + 1) * 128],
                        wvt[:, d * 256:(d + 1) * 256],
                        start=(d == 0), stop=(d == 7),
                    )
                vdst = vtb[b][:].rearrange(
                    "p (t g w c) -> p t g w c", t=4, g=NKV, w=2
                )[:, tki, :, 0, :]
                nc.vector.tensor_copy(vdst, ps[:, 0:NKV * HD])

        # Q^T: both batches in one N=512 matmul per (m, d); query columns
        # of X^T sit at cols [QB, 2*QB) within each batch's KW-wide block
        for m in range(8):
            ps = psA.tile([128, 512], F32, tag="ys", name=f"qps{m}")
            for d in range(8):
                rhs = xt[d][:].rearrange(
                    "p (b c) -> p b c", b=B
                )[:, :, KW - QB:KW]
                nc.tensor.matmul(
                    ps[:],
                    wqt[:, d * 1024 + m * 128:d * 1024 + (m + 1) * 128],
                    rhs,
                    start=(d == 0), stop=(d == 7),
                )
            nc.vector.tensor_copy(qt[m][:], ps[:])

        # ---------------- stage 3: attention per (batch, head-pair)
        pair_i = 0
        for b in range(B):
            for g in range(NKV):
                mk = g // 2          # K^T tile index
                kb = (g % 2) * 64    # partition base of this kv head
                for e2 in range(2):
                    h0 = 4 * g + 2 * e2
                    mqs = [HEAD_POS[h0] // 2, HEAD_POS[h0 + 1] // 2]

                    # window scores PSUM [128, 2048]; sink scores in snk
                    sp = psS.tile([128, SPWIN], F32, tag="s", name=f"s{b}{g}{e2}")
                    snk = psB.tile([128, 512], F32, tag="k", name=f"k{b}{g}{e2}")
                    for e in range(2):
                        qrhs = qt[mqs[e]][kb:kb + 64, b * QB:(b + 1) * QB]
                        for tki in range(4):
                            nc.tensor.matmul(
                                sp[:, tki * 512 + e * QB:tki * 512 + (e + 1) * QB],
                                kt[mk][kb:kb + 64,
                                       b * KW + tki * 128:b * KW + (tki + 1) * 128],
                                qrhs,
                                start=True, stop=True,
                            )
                        nc.tensor.matmul(
                            snk[:, e * QB:(e + 1) * QB],
                            ktp[(mk, b)][kb:kb + 64, :],
                            qrhs,
                            start=True, stop=True,
                        )

                    p = ppool.tile([128, SPW], MM, tag="p", name=f"p{b}{g}{e2}")
                    nc.scalar.activation(p[:, 0:SPWIN], sp[:], AF.Exp)
                    nc.scalar.activation(p[:, SPWIN:SPW], snk[:], AF.Exp)
                    nc.vector.tensor_mul(p[:], p[:], mtw[:])

                    # PV: V_aug = [V | ones*64] -> rows 0:64 Y, 64:128 denom
                    ys = psA.tile([128, 512], F32, tag="ys", name=f"ys{b}{g}{e2}")
                    for e in range(2):
                        for tki in range(4):
                            nc.tensor.matmul(
                                ys[:, e * QB:(e + 1) * QB],
                                vtb[b][:, tki * 512 + g * 128:tki * 512 + (g + 1) * 128],
                                p[:, tki * 512 + e * QB:tki * 512 + (e + 1) * QB],
                                start=(tki == 0), stop=False,
                            )
                        nc.tensor.matmul(
                            ys[:, e * QB:(e + 1) * QB],
                            vs[b][0:SINK, g * 128:(g + 1) * 128],
                            p[0:SINK, 8 * QB + e * QB:8 * QB + (e + 1) * QB],
                            start=False, stop=True,
                        )

                    # denominator: row 64 of ys -> SBUF (same-base copy) ->
                    # K=1 matmul broadcasts it to partitions 0:64 (into the
                    # recycled snk bank) -> reciprocal at base 0
                    dn = spool.tile([128, 512], mybir.dt.float32r, tag="dn",
                                    name=f"dn{b}{g}{e2}")
                    nc.vector.tensor_copy(dn[64:65, :], ys[64:65, :])
                    nc.tensor.matmul(
                        snk[0:64, :], ones[64:65, :], dn[64:65, :],
                        start=True, stop=True,
                    )
                    rb = spool.tile([64, 512], F32, tag="rb", name=f"rb{b}{g}{e2}")
                    if USE_FAST_RECIP:
                        nc.vector.reciprocal_approx_fast(rb[:], snk[0:64, :])
                    else:
                        nc.vector.reciprocal(rb[:], snk[0:64, :])
                    stg = spool.tile([64, 512], MM, tag="stg",
                                     name=f"stg{b}{g}{e2}")
                    nc.vector.tensor_mul(stg[:], ys[0:HD, :], rb[:])
                    for e in range(2):
                        mq = mqs[e]
                        nc.sync.dma_start(
                            yt[mq][kb:kb + 64, b * QB:(b + 1) * QB],
                            stg[:, e * QB:(e + 1) * QB],
                        )
                    pair_i += 1

        # ---------------- stage 4: O projection
        for b in range(B):
            for mq2 in range(2):
                for nk in range(2):
                    po = psA.tile([128, 512], F32, tag="ys", name=f"po{b}{mq2}{nk}")
                    for m in range(8):
                        nc.tensor.matmul(
                            po[:],
                            yt[m][:, b * QB + mq2 * 128:b * QB + (mq2 + 1) * 128],
                            wot[:, m * 1024 + nk * 512:m * 1024 + (nk + 1) * 512],
                            start=(m == 0), stop=(m == 7),
                        )
                    ost = opool.tile([128, 512], MM, tag="ost", name=f"o{b}{mq2}{nk}")
                    nc.vector.tensor_copy(ost[:], po[:])
                    nc.sync.dma_start(
                        out_d[b, mq2 * 128:(mq2 + 1) * 128, nk * 512:(nk + 1) * 512],
                        ost[:],
                    )

    nc.compile()
    return nc


# ================================================================ host side
def host_prep(X, Wq, Wk, Wv, Wo):
    """Returns in_maps (list of per-core dicts of numpy arrays)."""
    import ml_dtypes
    np_mm = np.dtype(ml_dtypes.bfloat16)

    X = np.asarray(X, dtype=np.float32)
    Wq = np.asarray(Wq, dtype=np.float32)
    Wk = np.asarray(Wk, dtype=np.float32)
    Wv = np.asarray(Wv, dtype=np.float32)
    Wo = np.asarray(Wo, dtype=np.float32)

    flat_perm = np.concatenate(
        [np.arange(h * HD, (h + 1) * HD) for h in HEAD_ORDER]
    )
    wq_p = (np.ascontiguousarray(Wq[:, flat_perm])
            * np.float32(1.0 / np.sqrt(HD)))
    wo_p = np.ascontiguousarray(Wo[flat_perm, :])

    # pack weights into [128, 8*cols]: partition p col-block d = rows d*128+p
    def pack(w):
        dd, cc = w.shape
        return np.ascontiguousarray(
            w.reshape(8, 128, cc).transpose(1, 0, 2).reshape(128, 8 * cc)
        ).astype(np_mm)

    wq_sb = pack(wq_p)
    wk_sb = pack(Wk)
    wv_sb = pack(Wv)
    wo_sb = pack(wo_p)

    # sink K^T / V_aug (host-computed, like the masks)
    Xs = X[:, 0:SINK, :]                       # [B, 4, D]
    Ks = Xs @ Wk                               # [B, 4, 256]
    Vsk = Xs @ Wv                              # [B, 4, 256]
    ktp_h = np.zeros((B, 2, 128, SINK), dtype=np.float32)
    vs_h = np.zeros((B, SINK, NKV * 128), dtype=np.float32)
    for b in range(B):
        for m in range(2):
            ktp_h[b, m] = Ks[b][:, m * 128:(m + 1) * 128].T
        for g in range(NKV):
            vs_h[b, :, g * 128:g * 128 + HD] = Vsk[b][:, g * HD:(g + 1) * HD]
            vs_h[b, :, g * 128 + HD:(g + 1) * 128] = 1.0

    tt = np.arange(T)
    i = tt[:, None]
    j = tt[None, :]
    m_full = (j <= i) & ((j < SINK) | (j >= np.maximum(i - WIN + 1, 0)))
    m_full = m_full.astype(np.float32)

    Xb = X.astype(np_mm)

    in_maps = []
    for c in range(NCORES):
        qs = c * QB
        ks = qs - QB  # window starts one query-block earlier (512 rows)

        xw = np.zeros((B, KW, D), dtype=np_mm)
        lo = max(ks, 0)
        xw[:, lo - ks:, :] = Xb[:, lo:ks + KW, :]

        # window mask, transposed: [key 512, query 256] -> [128, 4*256]
        mtw = np.zeros((KW, QB), dtype=np.float32)
        jg = ks + np.arange(KW)
        valid = jg >= 0
        mtw[valid, :] = m_full[qs:qs + QB, jg[valid]].T

        # sink mask [4, 256]; zero where the window tiles already cover col j
        mts = np.zeros((SINK, QB), dtype=np.float32)
        for jj in range(SINK):
            if not (ks <= jj < ks + KW):
                mts[jj, :] = m_full[qs:qs + QB, jj]

        # pair-merged mask [128, 10*QB]: cols tki*512 + e*256 + q (window,
        # same for both heads), 2048 + e*256 + q (sink, rows 4:128 zero)
        mtw2 = np.zeros((128, SPW), dtype=np.float32)
        wm = mtw.reshape(4, 128, QB)
        for tki in range(4):
            for e in range(2):
                mtw2[:, tki * 512 + e * QB:tki * 512 + (e + 1) * QB] = wm[tki]
        for e in range(2):
            mtw2[0:SINK, 8 * QB + e * QB:8 * QB + (e + 1) * QB] = mts

        in_maps.append({
            "ZER": np.zeros((128, 128), dtype=np_mm),
            "ONER": np.ones((128, 64), dtype=np.float32),
            "ONEB": np.ones((128, 4 * NKV * HD), dtype=np_mm),
            "Xw": xw,
            "Wq": wq_sb,
            "Wk": wk_sb,
            "Wv": wv_sb,
            "Wo": wo_sb,
            "MTw": mtw2.astype(np_mm),
            "KTP": ktp_h.astype(np_mm),
            "VS": vs_h.astype(np_mm),
        })
    return in_maps


_NC_CACHE = {}


def get_nc():
    if "nc" not in _NC_CACHE:
        _NC_CACHE["nc"] = build_nc()
    return _NC_CACHE["nc"]


def kernel(X, Wq, Wk, Wv, Wo):
    in_maps = host_prep(X, Wq, Wk, Wv, Wo)
    nc = get_nc()
    res = run_bass_kernel_spmd(nc, in_maps, list(range(NCORES)))
    out = np.empty((B, T, D), dtype=np.float32)
    for c in range(NCORES):
        out[:, c * QB:(c + 1) * QB, :] = res.results[c]["out"].astype(np.float32)
    return out
